# revision 19
# baseline (speedup 1.0000x reference)
"""DiffusionGPT Trainium2 kernel.

Data-parallel over batch: 8 batch elements -> 8 NeuronCores, one full
sequence per core.  Activations are kept feature-major in SBUF
([feature partitions, token free-dim]) so every matmul uses natural
weight layout (lhsT = weight tile [k_in, m_out]) with zero activation
transposes in the main path.  All big matmuls run as float32r
(full-rate fp32 on the PE for N>=256).

Layer schedule is chunk-pipelined for engine overlap:
  LN1(c) -> QKV(c) fused per chunk; attention is chunk-outer
  (all heads on chunk c, then proj(c)+residual+LN2(c)), so the
  Act-engine-bound softmax exp of one chunk overlaps PE-bound work of
  the neighbouring chunks; MLP runs un-fused with gelu contiguous so
  the Act function table switches only twice per layer.

Shapes (hardcoded from the problem spec):
  B=8, T=1022, S=1024, E=512, H=8 heads, D=64, F=2048, L=4 layers.
"""

import sys

sys.path.insert(0, "/opt/trn_rl_repo")

from contextlib import ExitStack

import numpy as np

import concourse.bass as bass
import concourse.bacc as bacc
import concourse.tile as tile
from concourse import mybir
from concourse.bass_utils import run_bass_kernel_spmd
from concourse.masks import make_identity
from concourse import library_config

# The act-table-load pass maps each function to the FIRST table set that
# contains it, which puts Ln (natural_log) and Exp (exp_and_others) in
# different tables and forces two table loads per layernorm rstd chain.
# Dropping Ln/Exp from the sets that precede natural_log_exp_and_others
# makes both resolve to that one (real, hardware-valid) table id.
import concourse.bacc as _bacc_mod
from concourse.hw_specs import get_activation_tables as _real_gat


def _patched_gat(arch):
    tabs = _real_gat(arch)
    out = {}
    seen = False
    drop = {mybir.ActivationFunctionType.Exp, mybir.ActivationFunctionType.Ln}
    for name, s in tabs.items():
        if name == "natural_log_exp_and_others":
            seen = True
        out[name] = s if seen else (s - drop)
    return out


_bacc_mod.get_activation_tables = _patched_gat

F32 = mybir.dt.float32
F32R = mybir.dt.float32r
AF = mybir.ActivationFunctionType
ALU = mybir.AluOpType

B = 8
T = 1022
S = 1024          # T + 2 tokens
E = 512
H = 8
D = 64
F = 2048
L = 4
NT = E // 128     # 4 feature tiles
NTT = S // 128    # 8 token tiles
LN_EPS = 1e-5
SCALE = 1.0 / 8.0  # 1/sqrt(D)

CHUNKS = ((0, 512), (512, 1024))  # token chunks for matmul N


def r(ap):
    return ap


def rr(ap):
    return ap.bitcast(F32R)


def build_nc(num_layers=L, do_head=True):
    nc = bacc.Bacc("TRN2", target_bir_lowering=False, debug=False)

    # ---- DRAM I/O ----
    d_sa = nc.dram_tensor("state_actions", [T, 72], F32, kind="ExternalInput")
    d_goals = nc.dram_tensor("goals", [1, 3], F32, kind="ExternalInput")
    d_sigma = nc.dram_tensor("sigma", [1], F32, kind="ExternalInput")
    d_sigma_w = nc.dram_tensor("sigma_w", [1, E], F32, kind="ExternalInput")
    d_sigma_b = nc.dram_tensor("sigma_b", [E], F32, kind="ExternalInput")
    d_tok_w = nc.dram_tensor("tok_w", [72, E], F32, kind="ExternalInput")
    d_tok_b = nc.dram_tensor("tok_b", [E], F32, kind="ExternalInput")
    d_goal_w = nc.dram_tensor("goal_w", [3, E], F32, kind="ExternalInput")
    d_goal_b = nc.dram_tensor("goal_b", [E], F32, kind="ExternalInput")
    d_pos = nc.dram_tensor("pos_emb", [1, S, E], F32, kind="ExternalInput")
    d_ln1_g = nc.dram_tensor("ln1_g", [L, E], F32, kind="ExternalInput")
    d_ln1_b = nc.dram_tensor("ln1_b", [L, E], F32, kind="ExternalInput")
    d_q_w = nc.dram_tensor("q_w", [L, E, E], F32, kind="ExternalInput")
    d_q_b = nc.dram_tensor("q_b", [L, E], F32, kind="ExternalInput")
    d_k_w = nc.dram_tensor("k_w", [L, E, E], F32, kind="ExternalInput")
    d_k_b = nc.dram_tensor("k_b", [L, E], F32, kind="ExternalInput")
    d_v_w = nc.dram_tensor("v_w", [L, E, E], F32, kind="ExternalInput")
    d_v_b = nc.dram_tensor("v_b", [L, E], F32, kind="ExternalInput")
    d_proj_w = nc.dram_tensor("proj_w", [L, E, E], F32, kind="ExternalInput")
    d_proj_b = nc.dram_tensor("proj_b", [L, E], F32, kind="ExternalInput")
    d_ln2_g = nc.dram_tensor("ln2_g", [L, E], F32, kind="ExternalInput")
    d_ln2_b = nc.dram_tensor("ln2_b", [L, E], F32, kind="ExternalInput")
    d_w1 = nc.dram_tensor("mlp_w1", [L, E, F], F32, kind="ExternalInput")
    d_b1 = nc.dram_tensor("mlp_b1", [L, F], F32, kind="ExternalInput")
    d_w2 = nc.dram_tensor("mlp_w2", [L, F, E], F32, kind="ExternalInput")
    d_b2 = nc.dram_tensor("mlp_b2", [L, E], F32, kind="ExternalInput")
    d_lnf_g = nc.dram_tensor("lnf_g", [E], F32, kind="ExternalInput")
    d_lnf_b = nc.dram_tensor("lnf_b", [E], F32, kind="ExternalInput")
    d_pred_w = nc.dram_tensor("pred_w", [E, 72], F32, kind="ExternalInput")
    d_pred_b = nc.dram_tensor("pred_b", [72], F32, kind="ExternalInput")
    d_out = nc.dram_tensor("out", [T, 72], F32, kind="ExternalOutput")

    with tile.TileContext(nc) as tc, ExitStack() as ctx:
        nc.gpsimd.load_library(library_config.attnmlp)

        const = ctx.enter_context(tc.tile_pool(name="const", bufs=1))
        big = ctx.enter_context(tc.tile_pool(name="big", bufs=1))
        wbig = ctx.enter_context(tc.tile_pool(name="wbig", bufs=12))
        vw1p = ctx.enter_context(tc.tile_pool(name="vw1p", bufs=4))
        w2p = ctx.enter_context(tc.tile_pool(name="w2p", bufs=2))
        bmat = ctx.enter_context(tc.tile_pool(name="bmat", bufs=1))
        bcols = ctx.enter_context(tc.tile_pool(name="bcols", bufs=4))
        ptp = ctx.enter_context(tc.tile_pool(name="ptp", bufs=3))
        usp = ctx.enter_context(tc.tile_pool(name="usp", bufs=3))
        rowp = ctx.enter_context(tc.tile_pool(name="rowp", bufs=4))
        recp = ctx.enter_context(tc.tile_pool(name="recp", bufs=2))
        scr = ctx.enter_context(tc.tile_pool(name="scr", bufs=2))
        sqp = ctx.enter_context(tc.tile_pool(name="sqp", bufs=3))
        bcp = ctx.enter_context(tc.tile_pool(name="bcp", bufs=2))

        ps_mm = ctx.enter_context(tc.tile_pool(name="ps_mm", bufs=2, space="PSUM"))
        ps_u = ctx.enter_context(tc.tile_pool(name="ps_u", bufs=2, space="PSUM"))
        ps_tp = ctx.enter_context(tc.tile_pool(name="ps_tp", bufs=2, space="PSUM"))

        # ---- constants ----
        ident = const.tile([128, 128], F32)
        make_identity(nc, ident[:])
        # memset can't write f32r directly (invalid ISA); memset f32 scratch
        # then DVE-copy (f32 -> f32r is a rounding write the verifier accepts)
        ones64_f32 = const.tile([128, 64], F32)
        nc.gpsimd.memset(ones64_f32[:], 1.0)
        # ones_row kept as f32 (memset-able); matmul/DMA users bitcast to f32r
        ones_f32 = const.tile([1, 1024], F32)
        nc.gpsimd.memset(ones_f32[:], 1.0)
        ones_row = ones_f32[:].bitcast(F32R)
        inve_f32 = const.tile([128, 1], F32)
        nc.gpsimd.memset(inve_f32[:], 1.0 / E)
        inve_col = const.tile([128, 1], F32R)
        nc.vector.tensor_copy(inve_col[:], inve_f32[:])
        eps_col = const.tile([128, 1], F32)
        nc.gpsimd.memset(eps_col[:], LN_EPS)
        # causal masks: keep iff f - j >= 0 (diag tiles o<3, after qoff shift)
        # and iff f - 128 - j >= 0 (the o=3 wide tile, base -128)
        mask_a = const.tile([128, 128], F32R)
        nc.vector.tensor_copy(mask_a[:, 0:64], ones64_f32[:, 0:64])
        nc.vector.tensor_copy(mask_a[:, 64:128], ones64_f32[:, 0:64])
        nc.gpsimd.affine_select(out=mask_a[:], in_=mask_a[:],
                                compare_op=mybir.AluOpType.is_ge, fill=0.0,
                                base=0, pattern=[[1, 128]],
                                channel_multiplier=-1)
        mask_b = const.tile([128, 256], F32R)
        for mj in range(4):
            nc.vector.tensor_copy(mask_b[:, 64 * mj: 64 * mj + 64],
                                  ones64_f32[:, 0:64])
        nc.gpsimd.affine_select(out=mask_b[:], in_=mask_b[:],
                                compare_op=mybir.AluOpType.is_ge, fill=0.0,
                                base=-128, pattern=[[1, 256]],
                                channel_multiplier=-1)

        # persistent activation tiles (feature-major: [feat part, token free])
        x_t = [big.tile([128, S], F32R, name=f"x{i}") for i in range(NT)]
        h_t = [big.tile([128, S], F32R, name=f"h{i}") for i in range(NT)]
        q_t = [big.tile([128, S], F32R, name=f"qa{i}") for i in range(NT)]
        k_t = [big.tile([128, S], F32R, name=f"ka{i}") for i in range(NT)]
        # y (attention out) reuses q storage: y chunk-c writes land after
        # every q chunk-c score-matmul read for that head, and remaining q
        # reads touch the other chunk's columns only.
        y_t = q_t
        # v token-major, augmented with a ones column per head: per k-tile
        # [128 tokens, 8 heads x (64 dims + 1 ones)]
        vtok = [big.tile([128, H * 65], F32R, name=f"vtok{i}") for i in range(NTT)]
        for kt in range(NTT):
            vt3 = vtok[kt].rearrange("p (h c) -> p h c", c=65)
            nc.vector.tensor_copy(
                vt3[:, :, 64:65],
                ones64_f32.rearrange("p (b c) -> p b c", c=1)[:, 0:H, :])

        # =================================================================
        # Embedding: build x (feature-major), tokens 0=sigma, 1=goal, 2..=sa
        # =================================================================
        # saT: [72 sa-features + ones row, 1022 sa tokens]
        saT = const.tile([73, T], F32R)
        # row 72 = ones (K-augmentation); engines can't start at partition 72,
        # but DMA can write any partition range
        nc.sync.dma_start(saT[72:73, :], ones_row[:, 0:T])
        for tt in range(NTT):
            ntt = min(128, T - tt * 128)
            sa_tok = scr.tile([128, 72], F32, tag="sa_tok")
            nc.sync.dma_start(sa_tok[0:ntt, :], d_sa[tt * 128: tt * 128 + ntt, :])
            tp = ps_tp.tile([128, 128], F32, tag="tp")
            nc.tensor.matmul(tp[0:72, 0:ntt], sa_tok[0:ntt, 0:72],
                             ident[0:ntt, 0:ntt], is_transpose=True)
            nc.vector.tensor_copy(saT[0:72, tt * 128: tt * 128 + ntt], tp[0:72, 0:ntt])

        tokw_aug = const.tile([73, E], F32R)
        nc.sync.dma_start(tokw_aug[0:72, :], rr(d_tok_w[:, :]))
        nc.sync.dma_start(tokw_aug[72:73, :], rr(d_tok_b.ap().rearrange("(a e) -> a e", a=1)))

        # sigma & goal columns via one K=7 matmul per feature tile:
        # lhsT rows: [sigma_w; sigma_b; goal_w(3); goal_b; pos0]
        G_sb = const.tile([7, E], F32)
        nc.sync.dma_start(G_sb[0:1, :], d_sigma_w[:, :])
        nc.sync.dma_start(G_sb[1:2, :], d_sigma_b.ap().rearrange("(a e) -> a e", a=1))
        nc.sync.dma_start(G_sb[2:5, :], d_goal_w[:, :])
        nc.sync.dma_start(G_sb[5:6, :], d_goal_b.ap().rearrange("(a e) -> a e", a=1))
        nc.sync.dma_start(G_sb[6:7, :], d_pos.ap()[0, 0:1, :])

        sig_sb = const.tile([1, 1], F32)
        nc.sync.dma_start(sig_sb[:], d_sigma.ap().rearrange("(a e) -> a e", a=1))
        lns = const.tile([1, 1], F32)
        nc.scalar.activation(lns[:], sig_sb[:], AF.Ln)
        # build both rhs columns as a single partition-0 row then transpose:
        # free 0..6  = column 0 pattern [ln(sig)/4, 1, 0,0,0, 0, 0]
        # free 7..13 = column 1 pattern [0, 0, g0,g1,g2, 1, 1]
        sg_row = const.tile([1, 14], F32)
        nc.gpsimd.memset(sg_row[:], 0.0)
        nc.scalar.activation(sg_row[0:1, 0:1], lns[:], AF.Copy, scale=0.25)
        nc.gpsimd.memset(sg_row[0:1, 1:2], 1.0)
        g_row = const.tile([1, 3], F32)
        nc.sync.dma_start(g_row[:], d_goals[:, :])
        nc.vector.tensor_copy(sg_row[0:1, 9:12], g_row[:])
        nc.gpsimd.memset(sg_row[0:1, 12:14], 1.0)
        sg_rhs = const.tile([7, 2], F32)
        for col in range(2):
            gtp = ps_tp.tile([128, 128], F32, tag="tp")
            nc.tensor.matmul(gtp[0:7, 0:1], sg_row[0:1, col * 7:(col + 1) * 7],
                             ident[0:1, 0:1], is_transpose=True)
            nc.vector.tensor_copy(sg_rhs[:, col: col + 1], gtp[0:7, 0:1])

        for fc in range(NT):
            sg_ps = ps_tp.tile([128, 128], F32, tag="tp")
            nc.tensor.matmul(sg_ps[0:128, 0:2], G_sb[:, fc * 128:(fc + 1) * 128],
                             sg_rhs[:], start=True, stop=True)
            nc.scalar.activation(x_t[fc][:, 0:2], sg_ps[0:128, 0:2], AF.Copy)

        # sa tokens: xe = saT.T @ [tok_w; tok_b] + pos, in sa-frame, then
        # transpose into x columns 2..1023
        for tt in range(NTT):
            ntt = min(128, T - tt * 128)
            xe_ps = ps_u.tile([128, 512], F32, tag="u")
            nc.tensor.matmul(xe_ps[0:ntt, :], r(saT[:, tt * 128: tt * 128 + ntt]),
                             r(tokw_aug[:]), start=True, stop=True)
            pos_sb = ptp.tile([128, E], F32, tag="pT")
            nc.sync.dma_start(pos_sb[0:ntt, :],
                              d_pos.ap()[0, tt * 128 + 1: tt * 128 + 1 + ntt, :])
            xe_tok = ptp.tile([128, E], F32, tag="pT")
            nc.vector.tensor_add(xe_tok[0:ntt, :], xe_ps[0:ntt, :], pos_sb[0:ntt, :])
            for fc in range(NT):
                tp = ps_tp.tile([128, 128], F32, tag="tp")
                nc.tensor.matmul(tp[:, 0:ntt],
                                 xe_tok[0:ntt, fc * 128:(fc + 1) * 128],
                                 ident[0:ntt, 0:ntt], is_transpose=True)
                if fc < 2:
                    nc.vector.tensor_copy(
                        x_t[fc][:, 2 + tt * 128: 2 + tt * 128 + ntt],
                        tp[:, 0:ntt])
                else:
                    nc.scalar.activation(
                        x_t[fc][:, 2 + tt * 128: 2 + tt * 128 + ntt],
                        tp[:, 0:ntt], AF.Copy)

        # =================================================================
        # helpers
        # =================================================================
        def layernorm(src_t, dst_t, g_col, b_col, after_chunk=None):
            """dst = LN(src) feature-major, chunk at a time.

            var = E[x^2] - mean^2.  Work is split across DVE/Pool/Act to
            balance engines: squares and applies alternate DVE/Pool, the
            1/E scaling is folded into the stats-matmul lhsT column, and
            mean^2 runs as an Act Square straight from PSUM.
            """
            for c, (c0, c1) in enumerate(CHUNKS):
                sq = []
                for ti in range(NT):
                    sqt = sqp.tile([128, 512], F32R, tag="sq")
                    nc.scalar.activation(sqt[:], src_t[ti][:, c0:c1], AF.Square)
                    sq.append(sqt)
                s1 = ps_tp.tile([1, 512], F32, tag="tp")
                for ti in range(NT):
                    nc.tensor.matmul(s1[:], r(inve_col[:]), r(src_t[ti][:, c0:c1]),
                                     start=(ti == 0), stop=(ti == NT - 1))
                s2 = ps_tp.tile([1, 512], F32, tag="tp")
                for ti in range(NT):
                    nc.tensor.matmul(s2[:], r(inve_col[:]), r(sq[ti][:]),
                                     start=(ti == 0), stop=(ti == NT - 1))
                mean_row = rowp.tile([1, 512], F32, tag="rows")
                nc.scalar.activation(mean_row[:], s1[:], AF.Copy)
                msq = rowp.tile([1, 512], F32, tag="rows")
                nc.scalar.activation(msq[:], s1[:], AF.Square)
                var_row = rowp.tile([1, 512], F32, tag="rows")
                nc.vector.tensor_sub(var_row[:], s2[:], msq[:])
                lrow = rowp.tile([1, 512], F32, tag="rows")
                nc.scalar.activation(lrow[:], var_row[:], AF.Ln,
                                     bias=eps_col[0:1, :])
                rstd_row = rowp.tile([1, 512], F32, tag="rows")
                nc.scalar.activation(rstd_row[:], lrow[:], AF.Exp, scale=-0.5)
                mean_b = bcp.tile([128, 512], F32R, tag="bc")
                nc.gpsimd.partition_broadcast(mean_b[:], rr(mean_row[:]))
                rstd_b = bcp.tile([128, 512], F32R, tag="bc")
                nc.gpsimd.partition_broadcast(rstd_b[:], rr(rstd_row[:]))
                for ti in range(NT):
                    nc.vector.tensor_sub(dst_t[ti][:, c0:c1],
                                         src_t[ti][:, c0:c1], mean_b[:])
                    nc.vector.tensor_mul(dst_t[ti][:, c0:c1],
                                         dst_t[ti][:, c0:c1], rstd_b[:])
                    if ti < 2:
                        nc.scalar.activation(dst_t[ti][:, c0:c1],
                                             dst_t[ti][:, c0:c1], AF.Identity,
                                             scale=g_col[ti], bias=b_col[ti])
                    else:
                        nc.gpsimd.tensor_scalar(dst_t[ti][:, c0:c1],
                                                dst_t[ti][:, c0:c1],
                                                g_col[ti], b_col[ti],
                                                ALU.mult, ALU.add)
                if after_chunk is not None:
                    after_chunk(c)

        # =================================================================
        # Transformer layers
        # =================================================================
        for l in range(num_layers):
            # ---- per-layer bias/gain matrix -> feature-major columns ----
            # rows: 0 ln1_g, 1 ln1_b, 2 ln2_g, 3 ln2_b, 4 q_b, 5 k_b, 6 v_b,
            #       7 proj_b, 8 mlp_b2, 9..12 mlp_b1
            Bm = bmat.tile([13, E], F32, tag="B")
            nc.sync.dma_start(Bm[0:1, :], d_ln1_g.ap()[l: l + 1, :])
            nc.sync.dma_start(Bm[1:2, :], d_ln1_b.ap()[l: l + 1, :])
            nc.sync.dma_start(Bm[2:3, :], d_ln2_g.ap()[l: l + 1, :])
            nc.sync.dma_start(Bm[3:4, :], d_ln2_b.ap()[l: l + 1, :])
            nc.sync.dma_start(Bm[4:5, :], d_q_b.ap()[l: l + 1, :])
            nc.sync.dma_start(Bm[5:6, :], d_k_b.ap()[l: l + 1, :])
            nc.sync.dma_start(Bm[6:7, :], d_v_b.ap()[l: l + 1, :])
            nc.sync.dma_start(Bm[7:8, :], d_proj_b.ap()[l: l + 1, :])
            nc.sync.dma_start(Bm[8:9, :], d_b2.ap()[l: l + 1, :])
            nc.sync.dma_start(Bm[9:13, :],
                              d_b1.ap()[l: l + 1, :].rearrange("a (b e) -> (a b) e", e=E))
            # K=1 aug-matmul rows must sit at partition 0
            projb_row = bmat.tile([1, E], F32R, tag="pbrow")
            nc.sync.dma_start(projb_row[:], rr(d_proj_b.ap()[l: l + 1, :]))
            b2_row = bmat.tile([1, E], F32R, tag="b2row")
            nc.sync.dma_start(b2_row[:], rr(d_b2.ap()[l: l + 1, :]))
            bc_t = []
            for fc in range(NT):
                tp = ps_tp.tile([128, 128], F32, tag="tp")
                nc.tensor.matmul(tp[:, 0:13], Bm[:, fc * 128:(fc + 1) * 128],
                                 ident[0:13, 0:13], is_transpose=True)
                bct = bcols.tile([128, 13], F32, tag="bc")
                nc.vector.tensor_copy(bct[:], tp[:, 0:13])
                bc_t.append(bct)

            g1 = [bc_t[ti][:, 0:1] for ti in range(NT)]
            b1_ = [bc_t[ti][:, 1:2] for ti in range(NT)]
            g2 = [bc_t[ti][:, 2:3] for ti in range(NT)]
            b2_ = [bc_t[ti][:, 3:4] for ti in range(NT)]

            # ---- weights for QKV + proj, prefetched at layer start ----
            w_qkv = {}
            for name, dw in (("q", d_q_w), ("k", d_k_w), ("v", d_v_w),
                             ("p", d_proj_w)):
                w_sb = []
                for kc in range(NT):
                    wt = wbig.tile([128, E], F32R, tag="w")
                    nc.sync.dma_start(wt[:], rr(dw.ap()[l, kc * 128:(kc + 1) * 128, :]))
                    w_sb.append(wt)
                w_qkv[name] = w_sb

            vfull = [vw1p.tile([128, F], F32R, tag="vw1", name=f"vf{i}")
                     for i in range(NT)]
            v_t = [tv[:, 0:S] for tv in vfull]

            # ---- LN1 with QKV fused per chunk ----
            def qkv_chunk(c):
                c0, c1 = CHUNKS[c]
                for name, bidx, out_t in (("q", 4, q_t), ("k", 5, k_t),
                                          ("v", 6, v_t)):
                    w_sb = w_qkv[name]
                    for op in range(NT // 2):
                        ps2 = ps_mm.tile([128, 1024], F32, tag="mm")
                        for j in range(2):
                            ot = 2 * op + j
                            psv = ps2[:, j * 512:(j + 1) * 512]
                            for kc in range(NT):
                                nc.tensor.matmul(
                                    psv, r(w_sb[kc][:, ot * 128:(ot + 1) * 128]),
                                    r(h_t[kc][:, c0:c1]),
                                    start=(kc == 0), stop=(kc == NT - 1))
                        for j in range(2):
                            ot = 2 * op + j
                            psv = ps2[:, j * 512:(j + 1) * 512]
                            if name == "q":
                                nc.scalar.activation(
                                    out_t[ot][:, c0:c1], psv, AF.Identity,
                                    bias=bc_t[ot][:, bidx:bidx + 1])
                            else:
                                nc.vector.tensor_scalar(
                                    out_t[ot][:, c0:c1], psv,
                                    bc_t[ot][:, bidx:bidx + 1], None, ALU.add)

            layernorm(x_t, h_t, g1, b1_, after_chunk=qkv_chunk)

            # ---- v -> token-major vtok (with ones cols kept intact) ----
            for kt in range(NTT):
                for fc in range(NT):
                    tp = ps_tp.tile([128, 128], F32, tag="tp")
                    nc.tensor.matmul(tp[:], v_t[fc][:, kt * 128:(kt + 1) * 128].bitcast(F32),
                                     ident[:], is_transpose=True)
                    dst = vtok[kt][:, 130 * fc: 130 * fc + 130] \
                        .rearrange("p (h c) -> p h c", c=65)[:, :, 0:64]
                    if fc % 2 == 0:
                        nc.vector.tensor_copy(
                            dst, tp[:].rearrange("p (h c) -> p h c", c=64))
                    else:
                        nc.scalar.activation(
                            dst, tp[:].rearrange("p (h c) -> p h c", c=64),
                            AF.Copy)

            # ---- attention, chunk-outer; proj+LN2 fused per chunk ----
            for c, (c0, c1) in enumerate(CHUNKS):
                n_kt = 4 * (c + 1)
                for hd in range(H):
                    ht = hd // 2
                    hp = (hd % 2) * 64
                    q_h = q_t[ht][hp: hp + 64, :]
                    k_h = k_t[ht][hp: hp + 64, :]
                    y_ps = ps_u.tile([65, 512], F32, tag="u")
                    for pr in range(n_kt // 2):
                        # two key tiles share one 2-bank score psum and (when
                        # profitable) a single merged exp over both; garbage
                        # columns between the two valid spans are exp'd but
                        # never read by the y matmuls
                        info = []
                        s2 = ps_mm.tile([128, 1024], F32, tag="mm")
                        pt = ptp.tile([128, 1024], F32R, tag="pT")
                        for j in range(2):
                            kt = 2 * pr + j
                            o = kt - 4 * c
                            # keep matmul N >= 256 (fp32r full rate): for o=3
                            # the extra cols [256:384) are fully non-causal and
                            # get zeroed by a wider affine_select (base -128)
                            qoff = min(128 * o, 256) if o >= 0 else 0
                            nc.tensor.matmul(
                                s2[:, j * 512 + qoff: (j + 1) * 512],
                                r(k_h[:, kt * 128:(kt + 1) * 128]),
                                r(q_h[:, c0 + qoff:c1]),
                                start=True, stop=True)
                            info.append((j, kt, o, qoff))
                        if info[0][3] >= 256 and info[1][3] >= 256:
                            for j, kt, o, qoff in info:
                                nc.scalar.activation(
                                    pt[:, j * 512 + qoff: (j + 1) * 512],
                                    s2[:, j * 512 + qoff: (j + 1) * 512],
                                    AF.Exp, scale=SCALE)
                        else:
                            qa = info[0][3]
                            nc.scalar.activation(pt[:, qa:1024], s2[:, qa:1024],
                                                 AF.Exp, scale=SCALE)
                        for j, kt, o, qoff in info:
                            if o >= 0:
                                # zero where token < key index via a constant
                                # triangular mask on DVE (keeps Pool free for
                                # the rec_b broadcasts)
                                mw = 128 if o < 3 else 256
                                m = mask_a if o < 3 else mask_b
                                nc.vector.tensor_mul(
                                    pt[:, j * 512 + qoff: j * 512 + qoff + mw],
                                    pt[:, j * 512 + qoff: j * 512 + qoff + mw],
                                    m[:, 0:mw])
                            nc.tensor.matmul(
                                y_ps[:, qoff:512],
                                r(vtok[kt][:, 65 * hd: 65 * hd + 65]),
                                r(pt[:, j * 512 + qoff: (j + 1) * 512]),
                                start=(kt == 0), stop=(kt == n_kt - 1))
                    rec = recp.tile([1, 512], F32, tag="rr", bufs=2)
                    nc.vector.reciprocal(rec[:], y_ps[64:65, :])
                    rec_b = recp.tile([64, 512], F32, tag="rb", bufs=2)
                    nc.gpsimd.partition_broadcast(rec_b[:], rec[:])
                    nc.vector.tensor_mul(y_t[ht][hp: hp + 64, c0:c1],
                                         y_ps[0:64, :], rec_b[:])

                # ---- proj + residual for this chunk ----
                pw_sb = w_qkv["p"]
                for op in range(NT // 2):
                    ps2 = ps_mm.tile([128, 1024], F32, tag="mm")
                    for j in range(2):
                        ot = 2 * op + j
                        psv = ps2[:, j * 512:(j + 1) * 512]
                        for kc in range(NT):
                            nc.tensor.matmul(
                                psv, r(pw_sb[kc][:, ot * 128:(ot + 1) * 128]),
                                r(y_t[kc][:, c0:c1]), start=(kc == 0), stop=False)
                        nc.tensor.matmul(psv, r(projb_row[:, ot * 128:(ot + 1) * 128]),
                                         r(ones_row[:, 0:512]), start=False, stop=True)
                    for j in range(2):
                        ot = 2 * op + j
                        nc.vector.tensor_add(x_t[ot][:, c0:c1], x_t[ot][:, c0:c1],
                                             ps2[:, j * 512:(j + 1) * 512])

                # ---- LN2 for this chunk (h <- LN2(x)) ----
                sq = []
                for ti in range(NT):
                    sqt = sqp.tile([128, 512], F32R, tag="sq")
                    nc.scalar.activation(sqt[:], x_t[ti][:, c0:c1], AF.Square)
                    sq.append(sqt)
                s1 = ps_tp.tile([1, 512], F32, tag="tp")
                for ti in range(NT):
                    nc.tensor.matmul(s1[:], r(inve_col[:]), r(x_t[ti][:, c0:c1]),
                                     start=(ti == 0), stop=(ti == NT - 1))
                s2 = ps_tp.tile([1, 512], F32, tag="tp")
                for ti in range(NT):
                    nc.tensor.matmul(s2[:], r(inve_col[:]), r(sq[ti][:]),
                                     start=(ti == 0), stop=(ti == NT - 1))
                mean_row = rowp.tile([1, 512], F32, tag="rows")
                nc.scalar.activation(mean_row[:], s1[:], AF.Copy)
                msq = rowp.tile([1, 512], F32, tag="rows")
                nc.scalar.activation(msq[:], s1[:], AF.Square)
                var_row = rowp.tile([1, 512], F32, tag="rows")
                nc.vector.tensor_sub(var_row[:], s2[:], msq[:])
                lrow = rowp.tile([1, 512], F32, tag="rows")
                nc.scalar.activation(lrow[:], var_row[:], AF.Ln,
                                     bias=eps_col[0:1, :])
                rstd_row = rowp.tile([1, 512], F32, tag="rows")
                nc.scalar.activation(rstd_row[:], lrow[:], AF.Exp, scale=-0.5)
                mean_b = bcp.tile([128, 512], F32R, tag="bc")
                nc.gpsimd.partition_broadcast(mean_b[:], rr(mean_row[:]))
                rstd_b = bcp.tile([128, 512], F32R, tag="bc")
                nc.gpsimd.partition_broadcast(rstd_b[:], rr(rstd_row[:]))
                for ti in range(NT):
                    nc.vector.tensor_sub(h_t[ti][:, c0:c1],
                                         x_t[ti][:, c0:c1], mean_b[:])
                    nc.vector.tensor_mul(h_t[ti][:, c0:c1],
                                         h_t[ti][:, c0:c1], rstd_b[:])
                    if ti < 2:
                        nc.scalar.activation(h_t[ti][:, c0:c1],
                                             h_t[ti][:, c0:c1], AF.Identity,
                                             scale=g2[ti], bias=b2_[ti])
                    else:
                        nc.gpsimd.tensor_scalar(h_t[ti][:, c0:c1],
                                                h_t[ti][:, c0:c1],
                                                g2[ti], b2_[ti],
                                                ALU.mult, ALU.add)

            # ---- MLP (both chunks; gelu block contiguous in Act queue) ----
            w1_sb = []
            for kc in range(NT):
                wt = vw1p.tile([128, F], F32R, tag="vw1")
                nc.scalar.dma_start(wt[:], rr(d_w1.ap()[l, kc * 128:(kc + 1) * 128, :]))
                w1_sb.append(wt)

            for c, (c0, c1) in enumerate(CHUNKS):
                big_ps = [ps_mm.tile([128, 1024], F32, tag="mm", name=f"ops{i}")
                          for i in range(2)]
                out_ps = [big_ps[i // 2][:, (i % 2) * 512:(i % 2 + 1) * 512]
                          for i in range(NT)]
                for h16 in range(F // 128):
                    u_ps = ps_u.tile([128, 512], F32, tag="u")
                    for kc in range(NT):
                        nc.tensor.matmul(
                            u_ps[:], r(w1_sb[kc][:, h16 * 128:(h16 + 1) * 128]),
                            r(h_t[kc][:, c0:c1]),
                            start=(kc == 0), stop=(kc == NT - 1))
                    u_s = usp.tile([128, 512], F32R, tag="us")
                    b1col = bc_t[h16 % 4][:, 9 + h16 // 4: 10 + h16 // 4]
                    nc.scalar.activation(u_s[:], u_ps[:], AF.Gelu, bias=b1col)
                    w2t = w2p.tile([128, E], F32R, tag="w2")
                    nc.sync.dma_start(w2t[:], rr(d_w2.ap()[l, h16 * 128:(h16 + 1) * 128, :]))
                    for ot in range(NT):
                        nc.tensor.matmul(
                            out_ps[ot], r(w2t[:, ot * 128:(ot + 1) * 128]),
                            r(u_s[:]), start=(h16 == 0), stop=False)
                for ot in range(NT):
                    nc.tensor.matmul(out_ps[ot],
                                     r(b2_row[:, ot * 128:(ot + 1) * 128]),
                                     r(ones_row[:, 0:512]), start=False, stop=True)
                    nc.vector.tensor_add(x_t[ot][:, c0:c1], x_t[ot][:, c0:c1],
                                         out_ps[ot])

        # =================================================================
        # Final LN + prediction head + output transpose
        # =================================================================
        if do_head:
            B2 = bmat.tile([13, E], F32, tag="B")
            nc.sync.dma_start(B2[0:1, :], d_lnf_g.ap().rearrange("(a e) -> a e", a=1))
            nc.sync.dma_start(B2[1:2, :], d_lnf_b.ap().rearrange("(a e) -> a e", a=1))
            bcf_t = []
            for fc in range(NT):
                tp = ps_tp.tile([128, 128], F32, tag="tp")
                nc.tensor.matmul(tp[:, 0:2], B2[0:2, fc * 128:(fc + 1) * 128],
                                 ident[0:2, 0:2], is_transpose=True)
                bct = bcols.tile([128, 13], F32, tag="bc")
                nc.vector.tensor_copy(bct[:, 0:2], tp[:, 0:2])
                bcf_t.append(bct)
            gf = [bcf_t[ti][:, 0:1] for ti in range(NT)]
            bf = [bcf_t[ti][:, 1:2] for ti in range(NT)]
            pw_sb = []
            for kc in range(NT):
                wt = wbig.tile([128, 72], F32R, tag="pw", bufs=4)
                nc.sync.dma_start(wt[:], rr(d_pred_w.ap()[kc * 128:(kc + 1) * 128, :]))
                pw_sb.append(wt)
            pb_row = const.tile([1, 72], F32R)
            nc.sync.dma_start(pb_row[:], rr(d_pred_b.ap().rearrange("(a e) -> a e", a=1)))

            outT = saT[0:72, :]  # saT is dead after embedding; reuse its storage

            def pred_chunk(c):
                # pred token range aligned to the LN chunk: [2:512) / [512:1024)
                c0 = 2 if c == 0 else 512
                c1 = 512 if c == 0 else S
                n = c1 - c0
                ps = ps_u.tile([128, 512], F32, tag="u")
                for kc in range(NT):
                    nc.tensor.matmul(ps[0:72, 0:n], r(pw_sb[kc][:]),
                                     r(h_t[kc][:, c0:c1]), start=(kc == 0), stop=False)
                nc.tensor.matmul(ps[0:72, 0:n], r(pb_row[:]), r(ones_row[:, 0:n]),
                                 start=False, stop=True)
                nc.scalar.activation(outT[:, c0 - 2: c1 - 2], ps[0:72, 0:n], AF.Copy)

            # final LN with per-chunk prediction head fused in
            layernorm(x_t, h_t, gf, bf, after_chunk=pred_chunk)

            for tt in range(NTT):
                ntt = min(128, T - tt * 128)
                tp = ps_tp.tile([128, 128], F32, tag="tp")
                nc.tensor.matmul(tp[0:ntt, 0:72], outT[:, tt * 128: tt * 128 + ntt].bitcast(F32),
                                 ident[0:72, 0:72], is_transpose=True)
                o_sb = scr.tile([128, 72], F32, tag="sa_tok")
                nc.vector.tensor_copy(o_sb[0:ntt, :], tp[0:ntt, 0:72])
                nc.sync.dma_start(d_out.ap()[tt * 128: tt * 128 + ntt, :],
                                  o_sb[0:ntt, :])

    nc.compile()
    return nc


_NC_CACHE = None


def _get_nc():
    global _NC_CACHE
    if _NC_CACHE is None:
        _NC_CACHE = build_nc()
    return _NC_CACHE


WEIGHT_NAMES = [
    "sigma_w", "sigma_b", "tok_w", "tok_b", "goal_w", "goal_b", "pos_emb",
    "ln1_g", "ln1_b", "q_w", "q_b", "k_w", "k_b", "v_w", "v_b",
    "proj_w", "proj_b", "ln2_g", "ln2_b", "mlp_w1", "mlp_b1", "mlp_w2",
    "mlp_b2", "lnf_g", "lnf_b", "pred_w", "pred_b",
]


def make_in_maps(inputs):
    sa = np.asarray(inputs["state_actions"], np.float32)
    goals = np.asarray(inputs["goals"], np.float32)
    sigma = np.asarray(inputs["sigma"], np.float32)
    shared = {n: np.ascontiguousarray(np.asarray(inputs[n], np.float32))
              for n in WEIGHT_NAMES}
    in_maps = []
    for b in range(B):
        m = dict(shared)
        m["state_actions"] = np.ascontiguousarray(sa[b])
        m["goals"] = np.ascontiguousarray(goals[b])
        m["sigma"] = np.ascontiguousarray(sigma[b: b + 1])
        in_maps.append(m)
    return in_maps


def run_spmd(inputs, **kwargs):
    nc = _get_nc()
    res = run_bass_kernel_spmd(nc, make_in_maps(inputs), list(range(B)), **kwargs)
    out = np.stack([res.results[c]["out"] for c in range(B)], axis=0)
    return out.astype(np.float32), res


def kernel(**inputs):
    out, _ = run_spmd(inputs)
    return out


# revision 20
# speedup vs baseline: 1.0042x; 1.0042x over previous
"""DiffusionGPT Trainium2 kernel.

Data-parallel over batch: 8 batch elements -> 8 NeuronCores, one full
sequence per core.  Activations are kept feature-major in SBUF
([feature partitions, token free-dim]) so every matmul uses natural
weight layout (lhsT = weight tile [k_in, m_out]) with zero activation
transposes in the main path.  All big matmuls run as float32r
(full-rate fp32 on the PE for N>=256).

Layer schedule is chunk-pipelined for engine overlap:
  LN1(c) -> QKV(c) fused per chunk; attention is chunk-outer
  (all heads on chunk c, then proj(c)+residual+LN2(c)), so the
  Act-engine-bound softmax exp of one chunk overlaps PE-bound work of
  the neighbouring chunks; MLP runs un-fused with gelu contiguous so
  the Act function table switches only twice per layer.

Shapes (hardcoded from the problem spec):
  B=8, T=1022, S=1024, E=512, H=8 heads, D=64, F=2048, L=4 layers.
"""

import sys

sys.path.insert(0, "/opt/trn_rl_repo")

from contextlib import ExitStack

import numpy as np

import concourse.bass as bass
import concourse.bacc as bacc
import concourse.tile as tile
from concourse import mybir
from concourse.bass_utils import run_bass_kernel_spmd
from concourse.masks import make_identity
from concourse import library_config

# The act-table-load pass maps each function to the FIRST table set that
# contains it, which puts Ln (natural_log) and Exp (exp_and_others) in
# different tables and forces two table loads per layernorm rstd chain.
# Dropping Ln/Exp from the sets that precede natural_log_exp_and_others
# makes both resolve to that one (real, hardware-valid) table id.
import concourse.bacc as _bacc_mod
from concourse.hw_specs import get_activation_tables as _real_gat


def _patched_gat(arch):
    tabs = _real_gat(arch)
    out = {}
    seen = False
    drop = {mybir.ActivationFunctionType.Exp, mybir.ActivationFunctionType.Ln}
    for name, s in tabs.items():
        if name == "natural_log_exp_and_others":
            seen = True
        out[name] = s if seen else (s - drop)
    return out


_bacc_mod.get_activation_tables = _patched_gat

F32 = mybir.dt.float32
F32R = mybir.dt.float32r
AF = mybir.ActivationFunctionType
ALU = mybir.AluOpType

B = 8
T = 1022
S = 1024          # T + 2 tokens
E = 512
H = 8
D = 64
F = 2048
L = 4
NT = E // 128     # 4 feature tiles
NTT = S // 128    # 8 token tiles
LN_EPS = 1e-5
SCALE = 1.0 / 8.0  # 1/sqrt(D)

CHUNKS = ((0, 512), (512, 1024))  # token chunks for matmul N


def r(ap):
    return ap


def rr(ap):
    return ap.bitcast(F32R)


def build_nc(num_layers=L, do_head=True):
    nc = bacc.Bacc("TRN2", target_bir_lowering=False, debug=False)

    # ---- DRAM I/O ----
    d_sa = nc.dram_tensor("state_actions", [T, 72], F32, kind="ExternalInput")
    d_goals = nc.dram_tensor("goals", [1, 3], F32, kind="ExternalInput")
    d_sigma = nc.dram_tensor("sigma", [1], F32, kind="ExternalInput")
    d_sigma_w = nc.dram_tensor("sigma_w", [1, E], F32, kind="ExternalInput")
    d_sigma_b = nc.dram_tensor("sigma_b", [E], F32, kind="ExternalInput")
    d_tok_w = nc.dram_tensor("tok_w", [72, E], F32, kind="ExternalInput")
    d_tok_b = nc.dram_tensor("tok_b", [E], F32, kind="ExternalInput")
    d_goal_w = nc.dram_tensor("goal_w", [3, E], F32, kind="ExternalInput")
    d_goal_b = nc.dram_tensor("goal_b", [E], F32, kind="ExternalInput")
    d_pos = nc.dram_tensor("pos_emb", [1, S, E], F32, kind="ExternalInput")
    d_ln1_g = nc.dram_tensor("ln1_g", [L, E], F32, kind="ExternalInput")
    d_ln1_b = nc.dram_tensor("ln1_b", [L, E], F32, kind="ExternalInput")
    d_q_w = nc.dram_tensor("q_w", [L, E, E], F32, kind="ExternalInput")
    d_q_b = nc.dram_tensor("q_b", [L, E], F32, kind="ExternalInput")
    d_k_w = nc.dram_tensor("k_w", [L, E, E], F32, kind="ExternalInput")
    d_k_b = nc.dram_tensor("k_b", [L, E], F32, kind="ExternalInput")
    d_v_w = nc.dram_tensor("v_w", [L, E, E], F32, kind="ExternalInput")
    d_v_b = nc.dram_tensor("v_b", [L, E], F32, kind="ExternalInput")
    d_proj_w = nc.dram_tensor("proj_w", [L, E, E], F32, kind="ExternalInput")
    d_proj_b = nc.dram_tensor("proj_b", [L, E], F32, kind="ExternalInput")
    d_ln2_g = nc.dram_tensor("ln2_g", [L, E], F32, kind="ExternalInput")
    d_ln2_b = nc.dram_tensor("ln2_b", [L, E], F32, kind="ExternalInput")
    d_w1 = nc.dram_tensor("mlp_w1", [L, E, F], F32, kind="ExternalInput")
    d_b1 = nc.dram_tensor("mlp_b1", [L, F], F32, kind="ExternalInput")
    d_w2 = nc.dram_tensor("mlp_w2", [L, F, E], F32, kind="ExternalInput")
    d_b2 = nc.dram_tensor("mlp_b2", [L, E], F32, kind="ExternalInput")
    d_lnf_g = nc.dram_tensor("lnf_g", [E], F32, kind="ExternalInput")
    d_lnf_b = nc.dram_tensor("lnf_b", [E], F32, kind="ExternalInput")
    d_pred_w = nc.dram_tensor("pred_w", [E, 72], F32, kind="ExternalInput")
    d_pred_b = nc.dram_tensor("pred_b", [72], F32, kind="ExternalInput")
    d_out = nc.dram_tensor("out", [T, 72], F32, kind="ExternalOutput")

    with tile.TileContext(nc) as tc, ExitStack() as ctx:
        nc.gpsimd.load_library(library_config.attnmlp)

        const = ctx.enter_context(tc.tile_pool(name="const", bufs=1))
        big = ctx.enter_context(tc.tile_pool(name="big", bufs=1))
        wbig = ctx.enter_context(tc.tile_pool(name="wbig", bufs=12))
        vw1p = ctx.enter_context(tc.tile_pool(name="vw1p", bufs=4))
        w2p = ctx.enter_context(tc.tile_pool(name="w2p", bufs=2))
        bmat = ctx.enter_context(tc.tile_pool(name="bmat", bufs=1))
        bcols = ctx.enter_context(tc.tile_pool(name="bcols", bufs=4))
        ptp = ctx.enter_context(tc.tile_pool(name="ptp", bufs=3))
        usp = ctx.enter_context(tc.tile_pool(name="usp", bufs=3))
        rowp = ctx.enter_context(tc.tile_pool(name="rowp", bufs=4))
        recp = ctx.enter_context(tc.tile_pool(name="recp", bufs=2))
        scr = ctx.enter_context(tc.tile_pool(name="scr", bufs=2))
        sqp = ctx.enter_context(tc.tile_pool(name="sqp", bufs=4))
        bcp = ctx.enter_context(tc.tile_pool(name="bcp", bufs=2))

        ps_mm = ctx.enter_context(tc.tile_pool(name="ps_mm", bufs=2, space="PSUM"))
        ps_u = ctx.enter_context(tc.tile_pool(name="ps_u", bufs=2, space="PSUM"))
        ps_tp = ctx.enter_context(tc.tile_pool(name="ps_tp", bufs=2, space="PSUM"))

        # ---- constants ----
        ident = const.tile([128, 128], F32)
        make_identity(nc, ident[:])
        # memset can't write f32r directly (invalid ISA); memset f32 scratch
        # then DVE-copy (f32 -> f32r is a rounding write the verifier accepts)
        ones64_f32 = const.tile([128, 64], F32)
        nc.gpsimd.memset(ones64_f32[:], 1.0)
        # ones_row kept as f32 (memset-able); matmul/DMA users bitcast to f32r
        ones_f32 = const.tile([1, 1024], F32)
        nc.gpsimd.memset(ones_f32[:], 1.0)
        ones_row = ones_f32[:].bitcast(F32R)
        inve_f32 = const.tile([128, 1], F32)
        nc.gpsimd.memset(inve_f32[:], 1.0 / E)
        inve_col = const.tile([128, 1], F32R)
        nc.vector.tensor_copy(inve_col[:], inve_f32[:])
        eps_col = const.tile([128, 1], F32)
        nc.gpsimd.memset(eps_col[:], LN_EPS)

        # persistent activation tiles (feature-major: [feat part, token free])
        x_t = [big.tile([128, S], F32R, name=f"x{i}") for i in range(NT)]
        h_t = [big.tile([128, S], F32R, name=f"h{i}") for i in range(NT)]
        q_t = [big.tile([128, S], F32R, name=f"qa{i}") for i in range(NT)]
        k_t = [big.tile([128, S], F32R, name=f"ka{i}") for i in range(NT)]
        # y (attention out) reuses q storage: y chunk-c writes land after
        # every q chunk-c score-matmul read for that head, and remaining q
        # reads touch the other chunk's columns only.
        y_t = q_t
        # v token-major, augmented with a ones column per head: per k-tile
        # [128 tokens, 8 heads x (64 dims + 1 ones)]
        vtok = [big.tile([128, H * 65], F32R, name=f"vtok{i}") for i in range(NTT)]
        for kt in range(NTT):
            vt3 = vtok[kt].rearrange("p (h c) -> p h c", c=65)
            nc.vector.tensor_copy(
                vt3[:, :, 64:65],
                ones64_f32.rearrange("p (b c) -> p b c", c=1)[:, 0:H, :])

        # =================================================================
        # Embedding: build x (feature-major), tokens 0=sigma, 1=goal, 2..=sa
        # =================================================================
        # saT: [72 sa-features + ones row, 1022 sa tokens]
        saT = const.tile([73, T], F32R)
        # row 72 = ones (K-augmentation); engines can't start at partition 72,
        # but DMA can write any partition range
        nc.sync.dma_start(saT[72:73, :], ones_row[:, 0:T])
        for tt in range(NTT):
            ntt = min(128, T - tt * 128)
            sa_tok = scr.tile([128, 72], F32, tag="sa_tok")
            nc.sync.dma_start(sa_tok[0:ntt, :], d_sa[tt * 128: tt * 128 + ntt, :])
            tp = ps_tp.tile([128, 128], F32, tag="tp")
            nc.tensor.matmul(tp[0:72, 0:ntt], sa_tok[0:ntt, 0:72],
                             ident[0:ntt, 0:ntt], is_transpose=True)
            nc.vector.tensor_copy(saT[0:72, tt * 128: tt * 128 + ntt], tp[0:72, 0:ntt])

        tokw_aug = const.tile([73, E], F32R)
        nc.sync.dma_start(tokw_aug[0:72, :], rr(d_tok_w[:, :]))
        nc.sync.dma_start(tokw_aug[72:73, :], rr(d_tok_b.ap().rearrange("(a e) -> a e", a=1)))

        # sigma & goal columns via one K=7 matmul per feature tile:
        # lhsT rows: [sigma_w; sigma_b; goal_w(3); goal_b; pos0]
        G_sb = const.tile([7, E], F32)
        nc.sync.dma_start(G_sb[0:1, :], d_sigma_w[:, :])
        nc.sync.dma_start(G_sb[1:2, :], d_sigma_b.ap().rearrange("(a e) -> a e", a=1))
        nc.sync.dma_start(G_sb[2:5, :], d_goal_w[:, :])
        nc.sync.dma_start(G_sb[5:6, :], d_goal_b.ap().rearrange("(a e) -> a e", a=1))
        nc.sync.dma_start(G_sb[6:7, :], d_pos.ap()[0, 0:1, :])

        sig_sb = const.tile([1, 1], F32)
        nc.sync.dma_start(sig_sb[:], d_sigma.ap().rearrange("(a e) -> a e", a=1))
        lns = const.tile([1, 1], F32)
        nc.scalar.activation(lns[:], sig_sb[:], AF.Ln)
        # build both rhs columns as a single partition-0 row then transpose:
        # free 0..6  = column 0 pattern [ln(sig)/4, 1, 0,0,0, 0, 0]
        # free 7..13 = column 1 pattern [0, 0, g0,g1,g2, 1, 1]
        sg_row = const.tile([1, 14], F32)
        nc.gpsimd.memset(sg_row[:], 0.0)
        nc.scalar.activation(sg_row[0:1, 0:1], lns[:], AF.Copy, scale=0.25)
        nc.gpsimd.memset(sg_row[0:1, 1:2], 1.0)
        g_row = const.tile([1, 3], F32)
        nc.sync.dma_start(g_row[:], d_goals[:, :])
        nc.vector.tensor_copy(sg_row[0:1, 9:12], g_row[:])
        nc.gpsimd.memset(sg_row[0:1, 12:14], 1.0)
        sg_rhs = const.tile([7, 2], F32)
        for col in range(2):
            gtp = ps_tp.tile([128, 128], F32, tag="tp")
            nc.tensor.matmul(gtp[0:7, 0:1], sg_row[0:1, col * 7:(col + 1) * 7],
                             ident[0:1, 0:1], is_transpose=True)
            nc.vector.tensor_copy(sg_rhs[:, col: col + 1], gtp[0:7, 0:1])

        for fc in range(NT):
            sg_ps = ps_tp.tile([128, 128], F32, tag="tp")
            nc.tensor.matmul(sg_ps[0:128, 0:2], G_sb[:, fc * 128:(fc + 1) * 128],
                             sg_rhs[:], start=True, stop=True)
            nc.scalar.activation(x_t[fc][:, 0:2], sg_ps[0:128, 0:2], AF.Copy)

        # sa tokens: xe = saT.T @ [tok_w; tok_b] + pos, in sa-frame, then
        # transpose into x columns 2..1023
        for tt in range(NTT):
            ntt = min(128, T - tt * 128)
            xe_ps = ps_u.tile([128, 512], F32, tag="u")
            nc.tensor.matmul(xe_ps[0:ntt, :], r(saT[:, tt * 128: tt * 128 + ntt]),
                             r(tokw_aug[:]), start=True, stop=True)
            pos_sb = ptp.tile([128, E], F32, tag="pT")
            nc.sync.dma_start(pos_sb[0:ntt, :],
                              d_pos.ap()[0, tt * 128 + 1: tt * 128 + 1 + ntt, :])
            xe_tok = ptp.tile([128, E], F32, tag="pT")
            nc.vector.tensor_add(xe_tok[0:ntt, :], xe_ps[0:ntt, :], pos_sb[0:ntt, :])
            for fc in range(NT):
                tp = ps_tp.tile([128, 128], F32, tag="tp")
                nc.tensor.matmul(tp[:, 0:ntt],
                                 xe_tok[0:ntt, fc * 128:(fc + 1) * 128],
                                 ident[0:ntt, 0:ntt], is_transpose=True)
                if fc < 2:
                    nc.vector.tensor_copy(
                        x_t[fc][:, 2 + tt * 128: 2 + tt * 128 + ntt],
                        tp[:, 0:ntt])
                else:
                    nc.scalar.activation(
                        x_t[fc][:, 2 + tt * 128: 2 + tt * 128 + ntt],
                        tp[:, 0:ntt], AF.Copy)

        # =================================================================
        # helpers
        # =================================================================
        def layernorm(src_t, dst_t, g_col, b_col, after_chunk=None):
            """dst = LN(src) feature-major, chunk at a time.

            var = E[x^2] - mean^2.  Work is split across DVE/Pool/Act to
            balance engines: squares and applies alternate DVE/Pool, the
            1/E scaling is folded into the stats-matmul lhsT column, and
            mean^2 runs as an Act Square straight from PSUM.
            """
            for c, (c0, c1) in enumerate(CHUNKS):
                sq = []
                for ti in range(NT):
                    sqt = sqp.tile([128, 512], F32R, tag="sq")
                    nc.scalar.activation(sqt[:], src_t[ti][:, c0:c1], AF.Square)
                    sq.append(sqt)
                s1 = ps_tp.tile([1, 512], F32, tag="tp")
                for ti in range(NT):
                    nc.tensor.matmul(s1[:], r(inve_col[:]), r(src_t[ti][:, c0:c1]),
                                     start=(ti == 0), stop=(ti == NT - 1))
                s2 = ps_tp.tile([1, 512], F32, tag="tp")
                for ti in range(NT):
                    nc.tensor.matmul(s2[:], r(inve_col[:]), r(sq[ti][:]),
                                     start=(ti == 0), stop=(ti == NT - 1))
                mean_row = rowp.tile([1, 512], F32, tag="rows")
                nc.scalar.activation(mean_row[:], s1[:], AF.Copy)
                msq = rowp.tile([1, 512], F32, tag="rows")
                nc.scalar.activation(msq[:], s1[:], AF.Square)
                var_row = rowp.tile([1, 512], F32, tag="rows")
                nc.vector.tensor_sub(var_row[:], s2[:], msq[:])
                lrow = rowp.tile([1, 512], F32, tag="rows")
                nc.scalar.activation(lrow[:], var_row[:], AF.Ln,
                                     bias=eps_col[0:1, :])
                rstd_row = rowp.tile([1, 512], F32, tag="rows")
                nc.scalar.activation(rstd_row[:], lrow[:], AF.Exp, scale=-0.5)
                mean_b = bcp.tile([128, 512], F32R, tag="bc")
                nc.gpsimd.partition_broadcast(mean_b[:], rr(mean_row[:]))
                rstd_b = bcp.tile([128, 512], F32R, tag="bc")
                nc.gpsimd.partition_broadcast(rstd_b[:], rr(rstd_row[:]))
                for ti in range(NT):
                    nc.vector.tensor_sub(dst_t[ti][:, c0:c1],
                                         src_t[ti][:, c0:c1], mean_b[:])
                    nc.vector.tensor_mul(dst_t[ti][:, c0:c1],
                                         dst_t[ti][:, c0:c1], rstd_b[:])
                    if ti < 2:
                        nc.scalar.activation(dst_t[ti][:, c0:c1],
                                             dst_t[ti][:, c0:c1], AF.Identity,
                                             scale=g_col[ti], bias=b_col[ti])
                    else:
                        nc.gpsimd.tensor_scalar(dst_t[ti][:, c0:c1],
                                                dst_t[ti][:, c0:c1],
                                                g_col[ti], b_col[ti],
                                                ALU.mult, ALU.add)
                if after_chunk is not None:
                    after_chunk(c)

        # =================================================================
        # Transformer layers
        # =================================================================
        for l in range(num_layers):
            # ---- per-layer bias/gain matrix -> feature-major columns ----
            # rows: 0 ln1_g, 1 ln1_b, 2 ln2_g, 3 ln2_b, 4 q_b, 5 k_b, 6 v_b,
            #       7 proj_b, 8 mlp_b2, 9..12 mlp_b1
            Bm = bmat.tile([13, E], F32, tag="B")
            nc.sync.dma_start(Bm[0:1, :], d_ln1_g.ap()[l: l + 1, :])
            nc.sync.dma_start(Bm[1:2, :], d_ln1_b.ap()[l: l + 1, :])
            nc.sync.dma_start(Bm[2:3, :], d_ln2_g.ap()[l: l + 1, :])
            nc.sync.dma_start(Bm[3:4, :], d_ln2_b.ap()[l: l + 1, :])
            nc.sync.dma_start(Bm[4:5, :], d_q_b.ap()[l: l + 1, :])
            nc.sync.dma_start(Bm[5:6, :], d_k_b.ap()[l: l + 1, :])
            nc.sync.dma_start(Bm[6:7, :], d_v_b.ap()[l: l + 1, :])
            nc.sync.dma_start(Bm[7:8, :], d_proj_b.ap()[l: l + 1, :])
            nc.sync.dma_start(Bm[8:9, :], d_b2.ap()[l: l + 1, :])
            nc.sync.dma_start(Bm[9:13, :],
                              d_b1.ap()[l: l + 1, :].rearrange("a (b e) -> (a b) e", e=E))
            # K=1 aug-matmul rows must sit at partition 0
            projb_row = bmat.tile([1, E], F32R, tag="pbrow")
            nc.sync.dma_start(projb_row[:], rr(d_proj_b.ap()[l: l + 1, :]))
            b2_row = bmat.tile([1, E], F32R, tag="b2row")
            nc.sync.dma_start(b2_row[:], rr(d_b2.ap()[l: l + 1, :]))
            bc_t = []
            for fc in range(NT):
                tp = ps_tp.tile([128, 128], F32, tag="tp")
                nc.tensor.matmul(tp[:, 0:13], Bm[:, fc * 128:(fc + 1) * 128],
                                 ident[0:13, 0:13], is_transpose=True)
                bct = bcols.tile([128, 13], F32, tag="bc")
                nc.vector.tensor_copy(bct[:], tp[:, 0:13])
                bc_t.append(bct)

            g1 = [bc_t[ti][:, 0:1] for ti in range(NT)]
            b1_ = [bc_t[ti][:, 1:2] for ti in range(NT)]
            g2 = [bc_t[ti][:, 2:3] for ti in range(NT)]
            b2_ = [bc_t[ti][:, 3:4] for ti in range(NT)]

            # ---- weights for QKV + proj, prefetched at layer start ----
            w_qkv = {}
            for name, dw in (("q", d_q_w), ("k", d_k_w), ("v", d_v_w),
                             ("p", d_proj_w)):
                w_sb = []
                for kc in range(NT):
                    wt = wbig.tile([128, E], F32R, tag="w")
                    nc.sync.dma_start(wt[:], rr(dw.ap()[l, kc * 128:(kc + 1) * 128, :]))
                    w_sb.append(wt)
                w_qkv[name] = w_sb

            vfull = [vw1p.tile([128, F], F32R, tag="vw1", name=f"vf{i}")
                     for i in range(NT)]
            v_t = [tv[:, 0:S] for tv in vfull]

            # ---- LN1 with QKV fused per chunk ----
            def qkv_chunk(c):
                c0, c1 = CHUNKS[c]
                for name, bidx, out_t in (("q", 4, q_t), ("k", 5, k_t),
                                          ("v", 6, v_t)):
                    w_sb = w_qkv[name]
                    for op in range(NT // 2):
                        ps2 = ps_mm.tile([128, 1024], F32, tag="mm")
                        for j in range(2):
                            ot = 2 * op + j
                            psv = ps2[:, j * 512:(j + 1) * 512]
                            for kc in range(NT):
                                nc.tensor.matmul(
                                    psv, r(w_sb[kc][:, ot * 128:(ot + 1) * 128]),
                                    r(h_t[kc][:, c0:c1]),
                                    start=(kc == 0), stop=(kc == NT - 1))
                        for j in range(2):
                            ot = 2 * op + j
                            psv = ps2[:, j * 512:(j + 1) * 512]
                            if name == "q":
                                nc.scalar.activation(
                                    out_t[ot][:, c0:c1], psv, AF.Identity,
                                    bias=bc_t[ot][:, bidx:bidx + 1])
                            else:
                                nc.vector.tensor_scalar(
                                    out_t[ot][:, c0:c1], psv,
                                    bc_t[ot][:, bidx:bidx + 1], None, ALU.add)

            layernorm(x_t, h_t, g1, b1_, after_chunk=qkv_chunk)

            # ---- v -> token-major vtok (with ones cols kept intact) ----
            for kt in range(NTT):
                for fc in range(NT):
                    tp = ps_tp.tile([128, 128], F32, tag="tp")
                    nc.tensor.matmul(tp[:], v_t[fc][:, kt * 128:(kt + 1) * 128].bitcast(F32),
                                     ident[:], is_transpose=True)
                    dst = vtok[kt][:, 130 * fc: 130 * fc + 130] \
                        .rearrange("p (h c) -> p h c", c=65)[:, :, 0:64]
                    if fc % 2 == 0:
                        nc.vector.tensor_copy(
                            dst, tp[:].rearrange("p (h c) -> p h c", c=64))
                    else:
                        nc.scalar.activation(
                            dst, tp[:].rearrange("p (h c) -> p h c", c=64),
                            AF.Copy)

            # ---- attention, chunk-outer; proj+LN2 fused per chunk ----
            for c, (c0, c1) in enumerate(CHUNKS):
                n_kt = 4 * (c + 1)
                for hd in range(H):
                    ht = hd // 2
                    hp = (hd % 2) * 64
                    q_h = q_t[ht][hp: hp + 64, :]
                    k_h = k_t[ht][hp: hp + 64, :]
                    y_ps = ps_u.tile([65, 512], F32, tag="u")
                    for pr in range(n_kt // 2):
                        # two key tiles share one 2-bank score psum and (when
                        # profitable) a single merged exp over both; garbage
                        # columns between the two valid spans are exp'd but
                        # never read by the y matmuls
                        info = []
                        s2 = ps_mm.tile([128, 1024], F32, tag="mm")
                        pt = ptp.tile([128, 1024], F32R, tag="pT")
                        for j in range(2):
                            kt = 2 * pr + j
                            o = kt - 4 * c
                            # keep matmul N >= 256 (fp32r full rate): for o=3
                            # the extra cols [256:384) are fully non-causal and
                            # get zeroed by a wider affine_select (base -128)
                            qoff = min(128 * o, 256) if o >= 0 else 0
                            nc.tensor.matmul(
                                s2[:, j * 512 + qoff: (j + 1) * 512],
                                r(k_h[:, kt * 128:(kt + 1) * 128]),
                                r(q_h[:, c0 + qoff:c1]),
                                start=True, stop=True)
                            info.append((j, kt, o, qoff))
                        if info[0][3] >= 256 and info[1][3] >= 256:
                            for j, kt, o, qoff in info:
                                nc.scalar.activation(
                                    pt[:, j * 512 + qoff: (j + 1) * 512],
                                    s2[:, j * 512 + qoff: (j + 1) * 512],
                                    AF.Exp, scale=SCALE)
                        else:
                            qa = info[0][3]
                            nc.scalar.activation(pt[:, qa:1024], s2[:, qa:1024],
                                                 AF.Exp, scale=SCALE)
                        for j, kt, o, qoff in info:
                            if o >= 0:
                                # zero where token < key index: keep iff
                                # (qoff + f) - (jj + 128*o) >= 0
                                mw = 128 if o < 3 else 256
                                nc.gpsimd.affine_select(
                                    out=pt[:, j * 512 + qoff: j * 512 + qoff + mw],
                                    in_=pt[:, j * 512 + qoff: j * 512 + qoff + mw],
                                    compare_op=mybir.AluOpType.is_ge, fill=0.0,
                                    base=qoff - 128 * o, pattern=[[1, mw]],
                                    channel_multiplier=-1)
                            nc.tensor.matmul(
                                y_ps[:, qoff:512],
                                r(vtok[kt][:, 65 * hd: 65 * hd + 65]),
                                r(pt[:, j * 512 + qoff: (j + 1) * 512]),
                                start=(kt == 0), stop=(kt == n_kt - 1))
                    rec = recp.tile([1, 512], F32, tag="rr", bufs=2)
                    nc.vector.reciprocal(rec[:], y_ps[64:65, :])
                    rec_b = recp.tile([64, 512], F32, tag="rb", bufs=2)
                    nc.gpsimd.partition_broadcast(rec_b[:], rec[:])
                    nc.vector.tensor_mul(y_t[ht][hp: hp + 64, c0:c1],
                                         y_ps[0:64, :], rec_b[:])

                # ---- proj + residual for this chunk ----
                pw_sb = w_qkv["p"]
                for op in range(NT // 2):
                    ps2 = ps_mm.tile([128, 1024], F32, tag="mm")
                    for j in range(2):
                        ot = 2 * op + j
                        psv = ps2[:, j * 512:(j + 1) * 512]
                        for kc in range(NT):
                            nc.tensor.matmul(
                                psv, r(pw_sb[kc][:, ot * 128:(ot + 1) * 128]),
                                r(y_t[kc][:, c0:c1]), start=(kc == 0), stop=False)
                        nc.tensor.matmul(psv, r(projb_row[:, ot * 128:(ot + 1) * 128]),
                                         r(ones_row[:, 0:512]), start=False, stop=True)
                    for j in range(2):
                        ot = 2 * op + j
                        nc.vector.tensor_add(x_t[ot][:, c0:c1], x_t[ot][:, c0:c1],
                                             ps2[:, j * 512:(j + 1) * 512])

                # ---- LN2 for this chunk (h <- LN2(x)) ----
                sq = []
                for ti in range(NT):
                    sqt = sqp.tile([128, 512], F32R, tag="sq")
                    nc.scalar.activation(sqt[:], x_t[ti][:, c0:c1], AF.Square)
                    sq.append(sqt)
                s1 = ps_tp.tile([1, 512], F32, tag="tp")
                for ti in range(NT):
                    nc.tensor.matmul(s1[:], r(inve_col[:]), r(x_t[ti][:, c0:c1]),
                                     start=(ti == 0), stop=(ti == NT - 1))
                s2 = ps_tp.tile([1, 512], F32, tag="tp")
                for ti in range(NT):
                    nc.tensor.matmul(s2[:], r(inve_col[:]), r(sq[ti][:]),
                                     start=(ti == 0), stop=(ti == NT - 1))
                mean_row = rowp.tile([1, 512], F32, tag="rows")
                nc.scalar.activation(mean_row[:], s1[:], AF.Copy)
                msq = rowp.tile([1, 512], F32, tag="rows")
                nc.scalar.activation(msq[:], s1[:], AF.Square)
                var_row = rowp.tile([1, 512], F32, tag="rows")
                nc.vector.tensor_sub(var_row[:], s2[:], msq[:])
                lrow = rowp.tile([1, 512], F32, tag="rows")
                nc.scalar.activation(lrow[:], var_row[:], AF.Ln,
                                     bias=eps_col[0:1, :])
                rstd_row = rowp.tile([1, 512], F32, tag="rows")
                nc.scalar.activation(rstd_row[:], lrow[:], AF.Exp, scale=-0.5)
                mean_b = bcp.tile([128, 512], F32R, tag="bc")
                nc.gpsimd.partition_broadcast(mean_b[:], rr(mean_row[:]))
                rstd_b = bcp.tile([128, 512], F32R, tag="bc")
                nc.gpsimd.partition_broadcast(rstd_b[:], rr(rstd_row[:]))
                for ti in range(NT):
                    nc.vector.tensor_sub(h_t[ti][:, c0:c1],
                                         x_t[ti][:, c0:c1], mean_b[:])
                    nc.vector.tensor_mul(h_t[ti][:, c0:c1],
                                         h_t[ti][:, c0:c1], rstd_b[:])
                    if ti < 2:
                        nc.scalar.activation(h_t[ti][:, c0:c1],
                                             h_t[ti][:, c0:c1], AF.Identity,
                                             scale=g2[ti], bias=b2_[ti])
                    else:
                        nc.gpsimd.tensor_scalar(h_t[ti][:, c0:c1],
                                                h_t[ti][:, c0:c1],
                                                g2[ti], b2_[ti],
                                                ALU.mult, ALU.add)

            # ---- MLP (both chunks; gelu block contiguous in Act queue) ----
            w1_sb = []
            for kc in range(NT):
                wt = vw1p.tile([128, F], F32R, tag="vw1")
                nc.scalar.dma_start(wt[:], rr(d_w1.ap()[l, kc * 128:(kc + 1) * 128, :]))
                w1_sb.append(wt)

            for c, (c0, c1) in enumerate(CHUNKS):
                big_ps = [ps_mm.tile([128, 1024], F32, tag="mm", name=f"ops{i}")
                          for i in range(2)]
                out_ps = [big_ps[i // 2][:, (i % 2) * 512:(i % 2 + 1) * 512]
                          for i in range(NT)]
                for h16 in range(F // 128):
                    u_ps = ps_u.tile([128, 512], F32, tag="u")
                    for kc in range(NT):
                        nc.tensor.matmul(
                            u_ps[:], r(w1_sb[kc][:, h16 * 128:(h16 + 1) * 128]),
                            r(h_t[kc][:, c0:c1]),
                            start=(kc == 0), stop=(kc == NT - 1))
                    u_s = usp.tile([128, 512], F32R, tag="us")
                    b1col = bc_t[h16 % 4][:, 9 + h16 // 4: 10 + h16 // 4]
                    nc.scalar.activation(u_s[:], u_ps[:], AF.Gelu, bias=b1col)
                    w2t = w2p.tile([128, E], F32R, tag="w2")
                    nc.sync.dma_start(w2t[:], rr(d_w2.ap()[l, h16 * 128:(h16 + 1) * 128, :]))
                    for ot in range(NT):
                        nc.tensor.matmul(
                            out_ps[ot], r(w2t[:, ot * 128:(ot + 1) * 128]),
                            r(u_s[:]), start=(h16 == 0), stop=False)
                for ot in range(NT):
                    nc.tensor.matmul(out_ps[ot],
                                     r(b2_row[:, ot * 128:(ot + 1) * 128]),
                                     r(ones_row[:, 0:512]), start=False, stop=True)
                    nc.vector.tensor_add(x_t[ot][:, c0:c1], x_t[ot][:, c0:c1],
                                         out_ps[ot])

        # =================================================================
        # Final LN + prediction head + output transpose
        # =================================================================
        if do_head:
            B2 = bmat.tile([13, E], F32, tag="B")
            nc.sync.dma_start(B2[0:1, :], d_lnf_g.ap().rearrange("(a e) -> a e", a=1))
            nc.sync.dma_start(B2[1:2, :], d_lnf_b.ap().rearrange("(a e) -> a e", a=1))
            bcf_t = []
            for fc in range(NT):
                tp = ps_tp.tile([128, 128], F32, tag="tp")
                nc.tensor.matmul(tp[:, 0:2], B2[0:2, fc * 128:(fc + 1) * 128],
                                 ident[0:2, 0:2], is_transpose=True)
                bct = bcols.tile([128, 13], F32, tag="bc")
                nc.vector.tensor_copy(bct[:, 0:2], tp[:, 0:2])
                bcf_t.append(bct)
            gf = [bcf_t[ti][:, 0:1] for ti in range(NT)]
            bf = [bcf_t[ti][:, 1:2] for ti in range(NT)]
            pw_sb = []
            for kc in range(NT):
                wt = wbig.tile([128, 72], F32R, tag="pw", bufs=4)
                nc.sync.dma_start(wt[:], rr(d_pred_w.ap()[kc * 128:(kc + 1) * 128, :]))
                pw_sb.append(wt)
            pb_row = const.tile([1, 72], F32R)
            nc.sync.dma_start(pb_row[:], rr(d_pred_b.ap().rearrange("(a e) -> a e", a=1)))

            outT = saT[0:72, :]  # saT is dead after embedding; reuse its storage

            def pred_chunk(c):
                # pred token range aligned to the LN chunk: [2:512) / [512:1024)
                c0 = 2 if c == 0 else 512
                c1 = 512 if c == 0 else S
                n = c1 - c0
                ps = ps_u.tile([128, 512], F32, tag="u")
                for kc in range(NT):
                    nc.tensor.matmul(ps[0:72, 0:n], r(pw_sb[kc][:]),
                                     r(h_t[kc][:, c0:c1]), start=(kc == 0), stop=False)
                nc.tensor.matmul(ps[0:72, 0:n], r(pb_row[:]), r(ones_row[:, 0:n]),
                                 start=False, stop=True)
                nc.scalar.activation(outT[:, c0 - 2: c1 - 2], ps[0:72, 0:n], AF.Copy)

            # final LN with per-chunk prediction head fused in
            layernorm(x_t, h_t, gf, bf, after_chunk=pred_chunk)

            for tt in range(NTT):
                ntt = min(128, T - tt * 128)
                tp = ps_tp.tile([128, 128], F32, tag="tp")
                nc.tensor.matmul(tp[0:ntt, 0:72], outT[:, tt * 128: tt * 128 + ntt].bitcast(F32),
                                 ident[0:72, 0:72], is_transpose=True)
                o_sb = scr.tile([128, 72], F32, tag="sa_tok")
                nc.vector.tensor_copy(o_sb[0:ntt, :], tp[0:ntt, 0:72])
                nc.sync.dma_start(d_out.ap()[tt * 128: tt * 128 + ntt, :],
                                  o_sb[0:ntt, :])

    nc.compile()
    return nc


_NC_CACHE = None


def _get_nc():
    global _NC_CACHE
    if _NC_CACHE is None:
        _NC_CACHE = build_nc()
    return _NC_CACHE


WEIGHT_NAMES = [
    "sigma_w", "sigma_b", "tok_w", "tok_b", "goal_w", "goal_b", "pos_emb",
    "ln1_g", "ln1_b", "q_w", "q_b", "k_w", "k_b", "v_w", "v_b",
    "proj_w", "proj_b", "ln2_g", "ln2_b", "mlp_w1", "mlp_b1", "mlp_w2",
    "mlp_b2", "lnf_g", "lnf_b", "pred_w", "pred_b",
]


def make_in_maps(inputs):
    sa = np.asarray(inputs["state_actions"], np.float32)
    goals = np.asarray(inputs["goals"], np.float32)
    sigma = np.asarray(inputs["sigma"], np.float32)
    shared = {n: np.ascontiguousarray(np.asarray(inputs[n], np.float32))
              for n in WEIGHT_NAMES}
    in_maps = []
    for b in range(B):
        m = dict(shared)
        m["state_actions"] = np.ascontiguousarray(sa[b])
        m["goals"] = np.ascontiguousarray(goals[b])
        m["sigma"] = np.ascontiguousarray(sigma[b: b + 1])
        in_maps.append(m)
    return in_maps


def run_spmd(inputs, **kwargs):
    nc = _get_nc()
    res = run_bass_kernel_spmd(nc, make_in_maps(inputs), list(range(B)), **kwargs)
    out = np.stack([res.results[c]["out"] for c in range(B)], axis=0)
    return out.astype(np.float32), res


def kernel(**inputs):
    out, _ = run_spmd(inputs)
    return out
